# revision 23
# baseline (speedup 1.0000x reference)
"""InterpretableMultiHeadAttention on 8 Trainium2 NeuronCores (Bass/Tile).

Sharding: core c -> batch b = c//2, head-group hg = c%2 (8 of 16 heads).
Math folding (exact up to fp rounding):
  v' = v @ Wv.T + bv, x = sum_h attn_h @ v'_h, out = x @ Wo.T + bo
  Since softmax rows sum to 1:  attn @ (1 bv^T) = 1 bv^T, so
  out = (sum_h attn_h @ v_h) @ (Wo @ Wv).T + (H * Wo @ bv + bo)
The 1/sqrt(d) score scale folds into Wq/bq.

Schedule per core: fp8 DoubleRow projections (contraction 256/step);
per query-block of 512: scores^T with 2-head row-packing (fp16, d=64
contraction via PE quadrants), exp split between ScalarE (table exp ->
fp8) and the DVE (one-op Schraudolph bit-trick exp -> fp8), PV matmuls
in fp8 DoubleRow over 256-key windows with a per-head ones-column at
64+h so softmax denominators accumulate on PSUM rows 64..71. Block
epilogue: 3-level den tree-sum, fast-approx reciprocal, PE-matmul
broadcast of 1/den to 64 partitions, divide + head tree-sum on DVE,
Wov projection, fp16 output DMA. Host sums the two half-head partial
projections per batch and adds the bias.
"""
import numpy as np
import ml_dtypes

N_OUT = 1024
N_HEADS = 16
D_K = 64
B = 4
S = 2048
FC8 = 4         # 1024 contraction as 4 DoubleRow chunks of 256
PAIRS = 4       # 8 local heads as 4 row-packed pairs
NMM = 512       # matmul moving free dim
JC = S // 128   # 16 key chunks of 128
JCP = JC // 2   # 8 key chunk-pairs of 256 (DoubleRow PV)
IQ = S // NMM   # 4 query blocks of 512
MV = 80         # PV lhsT cols: 64 v dims + ones at 64+h + pad to 16B step

# Schraudolph exp: fp8e4m3 bits = round(A8*s + B8); minimax C over [-3,3]
A8 = 8.0 * 1.4426950408889634
B8 = 56.0 - 0.370

# chunk (p, jc) exp routed to DVE iff (jc + p) % 3 == 0  (22 of 64)
def _dve_chunk(p, jc):
    return (jc + p) % 3 == 0

_CACHE = {}


def _build_nc(debug=False):
    from contextlib import ExitStack
    import concourse.bass as bass
    import concourse.bacc as bacc
    import concourse.tile as tile
    import concourse.mybir as mybir

    f16 = mybir.dt.float16
    f32 = mybir.dt.float32
    f32r = mybir.dt.float32r
    f8 = mybir.dt.float8e4
    i8 = mybir.dt.int8
    DR = mybir.MatmulPerfMode.DoubleRow

    nc = bacc.Bacc("TRN2", target_bir_lowering=False, debug=False, num_devices=8)

    xq_d = nc.dram_tensor("xq", [128, FC8, 2, S], f8, kind="ExternalInput")
    xk_d = nc.dram_tensor("xk", [128, FC8, 2, S], f8, kind="ExternalInput")
    wq_d = nc.dram_tensor("wq", [128, FC8, 2, 512], f8, kind="ExternalInput")
    wk_d = nc.dram_tensor("wk", [128, FC8, 2, 512], f8, kind="ExternalInput")
    bq_d = nc.dram_tensor("bq", [128, PAIRS], f32, kind="ExternalInput")
    bk_d = nc.dram_tensor("bk", [128, PAIRS], f32, kind="ExternalInput")
    vv_d = nc.dram_tensor("vv", [PAIRS, 128, JCP, 2, 2, MV], f8, kind="ExternalInput")
    wov_d = nc.dram_tensor("wov", [64, N_OUT], f16, kind="ExternalInput")
    sel_d = nc.dram_tensor("sel", [128, 8, 64], f16, kind="ExternalInput")
    out_d = nc.dram_tensor("outT", [128, 4, IQ, 2, NMM], f16,
                           kind="ExternalOutput")
    if debug:
        dbg_qT = nc.dram_tensor("dbg_qT", [128, 4, 512], f16, kind="ExternalOutput")
        dbg_kT = nc.dram_tensor("dbg_kT", [128, 4, 512], f16, kind="ExternalOutput")
        dbg_ep = nc.dram_tensor("dbg_ep", [128, 2, 2, NMM], f16, kind="ExternalOutput")
        dbg_yb = nc.dram_tensor("dbg_yb", [72, 8, NMM], f16, kind="ExternalOutput")
        dbg_dn = nc.dram_tensor("dbg_dn", [128, NMM], f32, kind="ExternalOutput")
        dbg_y16 = nc.dram_tensor("dbg_y16", [64, 512], f16, kind="ExternalOutput")

    with tile.TileContext(nc) as tc, ExitStack() as ctx:
        const = ctx.enter_context(tc.tile_pool(name="const", bufs=1))
        qkall = ctx.enter_context(tc.tile_pool(name="qkall", bufs=1))
        epool = ctx.enter_context(tc.tile_pool(name="epool", bufs=6))
        blk = ctx.enter_context(tc.tile_pool(name="blk", bufs=2))
        fin = ctx.enter_context(tc.tile_pool(name="fin", bufs=2))
        ost_p = ctx.enter_context(tc.tile_pool(name="ost_p", bufs=2))
        ps_mm = ctx.enter_context(tc.tile_pool(name="ps_mm", bufs=3, space="PSUM"))
        ps_y = ctx.enter_context(tc.tile_pool(name="ps_y", bufs=2, space="PSUM"))
        xctx = ExitStack()
        xstage = xctx.enter_context(tc.tile_pool(name="xstage", bufs=1))

        # ---- input loads (proj-critical first) ----
        wq_sb = const.tile([128, FC8, 2, 512], f8, tag="wq")
        nc.sync.dma_start(out=wq_sb[:], in_=wq_d[:])
        bq_sb = const.tile([128, PAIRS], f32, tag="bq")
        nc.sync.dma_start(out=bq_sb[:], in_=bq_d[:])
        bk_sb = const.tile([128, PAIRS], f32, tag="bk")
        nc.gpsimd.dma_start(out=bk_sb[:], in_=bk_d[:])
        wk_sb = const.tile([128, FC8, 2, 512], f8, tag="wk")
        nc.scalar.dma_start(out=wk_sb[:], in_=wk_d[:])
        engs = [nc.sync, nc.scalar, nc.gpsimd]
        xq_sb = xstage.tile([128, FC8, 2, S], f8, tag="xq")
        xk_sb = xstage.tile([128, FC8, 2, S], f8, tag="xk")
        ei = 0
        for fc in range(FC8):
            engs[ei % 3].dma_start(out=xq_sb[:, fc], in_=xq_d[:, fc])
            ei += 1
            engs[ei % 3].dma_start(out=xk_sb[:, fc], in_=xk_d[:, fc])
            ei += 1
        vv_sb = []
        for p in range(PAIRS):
            t = qkall.tile([128, JCP, 2, 2, MV], f8, tag=f"vv{p}")
            nc.sync.dma_start(out=t[:], in_=vv_d[p])
            vv_sb.append(t)
        wov_sb = const.tile([64, N_OUT], f16, tag="wov")
        nc.sync.dma_start(out=wov_sb[:], in_=wov_d[:])
        sel_sb = const.tile([128, 8, 64], f16, tag="sel")
        nc.gpsimd.dma_start(out=sel_sb[:], in_=sel_d[:])
        den32 = const.tile([128, NMM], f32, tag="den32")
        nc.vector.memset(den32[:], 1.0)
        rec32 = const.tile([128, NMM], f32, tag="rec32")

        qT, kT = {}, {}

        def _proj_quarter(p, qk, sc2):
            dst = (qT if qk == 0 else kT)[p]
            w_sb = wq_sb if qk == 0 else wk_sb
            b_sb = bq_sb if qk == 0 else bk_sb
            x_sb = xq_sb if qk == 0 else xk_sb
            ps = ps_mm.tile([128, 2, 512], f32, tag="mm", name="projps")
            for half in range(2):
                c0 = (sc2 * 2 + half) * 512
                for fc in range(FC8):
                    nc.tensor.matmul(
                        out=ps[:, half, :],
                        lhsT=w_sb[:, fc, :, p * 128:(p + 1) * 128],
                        rhs=x_sb[:, fc, :, c0:c0 + 512],
                        start=(fc == 0),
                        stop=(fc == FC8 - 1),
                        perf_mode=DR,
                    )
            if qk == 0:
                nc.scalar.add(
                    out=dst[:, sc2 * 2:sc2 * 2 + 2, :],
                    in_=ps[:],
                    add=b_sb[:, p:p + 1],
                )
            else:
                nc.vector.tensor_scalar_add(
                    out=dst[:, sc2 * 2:sc2 * 2 + 2, :],
                    in0=ps[:],
                    scalar1=b_sb[:, p:p + 1],
                )

        def proj_closures(p):
            qT[p] = qkall.tile([128, 4, 512], f16, tag=f"qT{p}", name=f"qT{p}")
            kT[p] = qkall.tile([128, 4, 512], f16, tag=f"kT{p}", name=f"kT{p}")
            return [lambda qk=qk, sc2=sc2: _proj_quarter(p, qk, sc2)
                    for sc2 in range(2) for qk in range(2)]

        def emit_fin_rb(st, p):
            """rb2 = PE broadcast of 1/den for heads 2p, 2p+1."""
            rb2 = ps_mm.tile([128, 2, 512], f32, tag="mm", name=f"rb{p}")
            for j in range(2):
                nc.tensor.matmul(
                    out=rb2[0:64, j, :],
                    lhsT=sel_sb[64:72, 2 * p + j, :],
                    rhs=st["den16"][64:72, :],
                    start=True, stop=True,
                    tile_position=(64, 0),
                )
            st[f"rb{p}"] = rb2

        def emit_fin_mul(st, p):
            if f"rb{p}" not in st:
                emit_fin_rb(st, p)
            rb2 = st[f"rb{p}"]
            m = fin.tile([64, 2, 512], f16, tag=f"m{p}", name=f"m{p}")
            if st["last"]:
                # tail path: stage 1/den via the otherwise-idle ScalarE so
                # the DVE multiply runs at 2x (both operands fp16 SBUF)
                rb16 = fin.tile([64, 2, 512], f16, tag=f"rb16_{p}",
                                name=f"rb16_{p}")
                nc.scalar.copy(out=rb16[:], in_=rb2[0:64, :, :])
                nc.vector.tensor_mul(
                    out=m[:], in0=st["y_blk"][0:64, 2 * p:2 * p + 2, :],
                    in1=rb16[:])
            else:
                nc.vector.tensor_mul(
                    out=m[:], in0=st["y_blk"][0:64, 2 * p:2 * p + 2, :],
                    in1=rb2[0:64, :, :])
            st[f"m{p}"] = m

        def emit_fin_add(st, k):
            t = fin.tile([64, 2, 512], f16, tag=f"t{k}", name=f"t{k}")
            nc.vector.tensor_add(
                out=t[:], in0=st[f"m{2 * k}"][:], in1=st[f"m{2 * k + 1}"][:])
            st[f"t{k}"] = t

        def emit_fin_y16(st):
            tt = fin.tile([64, 2, 512], f16, tag="tt")
            nc.vector.tensor_add(out=tt[:], in0=st["t0"][:], in1=st["t1"][:])
            y16 = blk.tile([64, 512], f16, tag="y16")
            nc.vector.tensor_add(out=y16[:], in0=tt[:, 0, :], in1=tt[:, 1, :])
            st["y16"] = y16

        def emit_out(st, m2):
            po = ps_mm.tile([128, 2, 512], f32, tag="mm", name=f"po{m2}")
            for j in range(2):
                nc.tensor.matmul(
                    out=po[:, j, :],
                    lhsT=wov_sb[:, (2 * m2 + j) * 128:(2 * m2 + j + 1) * 128],
                    rhs=st["y16"][:],
                    start=True, stop=True,
                )
            ost = ost_p.tile([128, 2, NMM], f16, tag="ost")
            if st["last"] and m2 % 2 == 0:
                nc.scalar.copy(out=ost[:], in_=po[:])
            else:
                nc.vector.tensor_copy(out=ost[:], in_=po[:])
            eng = nc.sync if m2 % 2 == 0 else nc.gpsimd
            eng.dma_start(out=out_d[:, m2, st["iq"], :, :], in_=ost[:])

        def fin_closures(st, drain=False):
            ops = []
            if drain:
                # front-load the PE broadcasts so the DVE divide chain
                # never waits on a matmul mid-stream
                ops += [lambda p=p: emit_fin_rb(st, p) for p in range(PAIRS)]
            ops += [lambda p=p: emit_fin_mul(st, p) for p in range(PAIRS)]
            ops += [lambda k=k: emit_fin_add(st, k) for k in range(2)]
            ops.append(lambda: emit_fin_y16(st))
            ops += [lambda m2=m2: emit_out(st, m2) for m2 in range(4)]
            return ops

        prev = None         # epilogue state dict of the previous block
        bops = []           # spread-out boundary ops, one per chunk-pair
        pend_pv = None      # deferred PV emission (one chunk-pair behind)
        for iq in range(IQ):
            y_blk = blk.tile([72, 8, NMM], f16, tag="yblk")
            den_acc = blk.tile([128, 2, NMM], f16, tag="dacc")
            for p in range(PAIRS):
                if iq == 0:
                    if p == 0:
                        p0ops = proj_closures(0)
                        for op in p0ops[:2]:
                            op()
                        bops = p0ops[2:] + proj_closures(1)
                    elif p < PAIRS - 1:
                        bops = bops + proj_closures(p + 1)
                if p == 1 and prev is not None:
                    bops = bops + fin_closures(prev)
                    prev = None
                yP = [ps_y.tile([MV, NMM], f32, tag="yab", name=f"yP{a}")
                      for a in range(2)]

                def emit_pv(p, jcp, ep, yP):
                    for a in range(2):
                        nc.tensor.matmul(
                            out=yP[a][:],
                            lhsT=vv_sb[p][:, jcp, a, :, :],
                            rhs=ep[:, a, :, :],
                            start=(jcp == 0), stop=(jcp == JCP - 1),
                            perf_mode=DR,
                            skip_group_check=True,
                        )
                    if jcp == JCP - 1:
                        # drain this pair: numerators + den rows to SBUF
                        for a in range(2):
                            nc.vector.tensor_copy(
                                out=y_blk[:, 2 * p + a, :], in_=yP[a][0:72, :])
                        if p == 0:
                            nc.vector.tensor_copy(
                                out=den_acc[64:72], in_=y_blk[64:72, 0:2, :])
                        else:
                            nc.vector.tensor_add(
                                out=den_acc[64:72], in0=den_acc[64:72],
                                in1=y_blk[64:72, 2 * p:2 * p + 2, :])

                for jcp in range(JCP):
                    ep = epool.tile([128, 2, 2, NMM], f8, tag="e")
                    for ko in range(2):
                        jc = 2 * jcp + ko
                        sAB = ps_mm.tile([128, 2, 512], f32, tag="mm")
                        nc.tensor.matmul(
                            out=sAB[:, 0, :],
                            lhsT=kT[p][0:64, jc // 4, (jc % 4) * 128:(jc % 4) * 128 + 128],
                            rhs=qT[p][0:64, iq, :],
                            start=True, stop=True,
                            tile_position=(0, 0),
                        )
                        nc.tensor.matmul(
                            out=sAB[:, 1, :],
                            lhsT=kT[p][64:128, jc // 4, (jc % 4) * 128:(jc % 4) * 128 + 128],
                            rhs=qT[p][64:128, iq, :],
                            start=True, stop=True,
                            tile_position=(64, 0),
                        )
                        if _dve_chunk(p, jc):
                            nc.vector.tensor_scalar(
                                out=ep.bitcast(i8)[:, :, ko, :],
                                in0=sAB[:],
                                scalar1=A8, scalar2=B8,
                                op0=mybir.AluOpType.mult,
                                op1=mybir.AluOpType.add,
                            )
                        else:
                            nc.scalar.activation(
                                out=ep[:, :, ko, :], in_=sAB[:],
                                func=mybir.ActivationFunctionType.Exp,
                            )
                    if pend_pv is not None:
                        pend_pv()
                    pend_pv = (lambda p=p, jcp=jcp, ep=ep, yP=yP:
                               emit_pv(p, jcp, ep, yP))
                    if bops:
                        bops.pop(0)()
            if pend_pv is not None:
                pend_pv()
                pend_pv = None
            if iq == 0:
                xctx.close()  # release x staging after last projection

            # block-end denominator fold + fast reciprocal (partitions 64..71;
            # reciprocal_approx_fast mishandles base_partition != 0, so it
            # runs full-tile over the once-memset background)
            nc.vector.tensor_add(
                out=den32[64:72], in0=den_acc[64:72, 0, :],
                in1=den_acc[64:72, 1, :])
            nc.vector.reciprocal_approx_fast(out=rec32[:, :], in_=den32[:, :])
            den16 = blk.tile([128, NMM], f16, tag="den16")
            nc.vector.tensor_copy(out=den16[64:72, :], in_=rec32[64:72, :])
            prev = {"iq": iq, "y_blk": y_blk, "den16": den16,
                    "last": iq == IQ - 1}
        # drain the last block
        for op in fin_closures(prev, drain=True):
            op()

    nc.compile()
    return nc


def _prep(queries, keys, values, Wq, bq, Wk, bk, Wv, bv, Wo, bo):
    """Host-side sharding/layout prep. Returns (in_maps, bo_p)."""
    queries = np.asarray(queries, np.float32)
    keys = np.asarray(keys, np.float32)
    values = np.asarray(values, np.float32)
    Wq = np.asarray(Wq, np.float32)
    bq = np.asarray(bq, np.float32)
    Wk = np.asarray(Wk, np.float32)
    bk = np.asarray(bk, np.float32)
    Wv = np.asarray(Wv, np.float32)
    bv = np.asarray(bv, np.float32)
    Wo = np.asarray(Wo, np.float32)
    bo = np.asarray(bo, np.float32)
    f8 = ml_dtypes.float8_e4m3

    scale = 1.0 / np.sqrt(np.float32(D_K))
    Wq_s = Wq * scale
    bq_s = bq * scale
    Wov = Wo @ Wv                       # [1024, 64]
    bo_p = bo + N_HEADS * (Wo @ bv)     # [1024]
    wov_h = np.ascontiguousarray(Wov.T.astype(np.float16))  # [64, 1024]

    # sel[64+r, h, :]: ones row used to broadcast recip row r=h to 64 parts
    sel = np.zeros((128, 8, 64), np.float16)
    for h in range(8):
        sel[64 + h, h, :] = 1.0

    def contr_fold(a):
        # [1024, cols] -> [128, 4, 2, cols]: contraction g = fc*256+j*128+k
        cols = a.shape[1]
        return np.ascontiguousarray(
            a.reshape(FC8, 2, 128, cols).transpose(2, 0, 1, 3).astype(f8))

    in_maps = []
    for c in range(8):
        b = c // 2
        hg = c % 2
        hsl = slice(hg * 512, (hg + 1) * 512)
        xq = contr_fold(queries[b].T)
        xk = contr_fold(keys[b].T)
        wq = contr_fold(Wq_s[hsl].T)
        wk = contr_fold(Wk[hsl].T)
        bq_c = np.ascontiguousarray(bq_s[hsl].reshape(PAIRS, 128).T)
        bk_c = np.ascontiguousarray(bk[hsl].reshape(PAIRS, 128).T)
        # vv [PAIRS, 128, JCP, 2(a), 2(ko), MV]; key = jcp*256 + ko*128 + k
        vb = values[b][:, hsl].reshape(JCP, 2, 128, 8, 64)  # [jcp, ko, k, h, d]
        vv = np.zeros((128, JCP, 8, 2, MV), f8)
        vv[:, :, :, :, :64] = vb.transpose(2, 0, 3, 1, 4).astype(f8)
        for h in range(8):
            vv[:, :, h, :, 64 + h] = 1.0
        vv = vv.reshape(128, JCP, PAIRS, 2, 2, MV).transpose(2, 0, 1, 3, 4, 5)
        in_maps.append({
            "xq": xq, "xk": xk, "wq": wq, "wk": wk,
            "bq": bq_c, "bk": bk_c, "vv": np.ascontiguousarray(vv),
            "wov": wov_h, "sel": sel,
        })
    return in_maps, bo_p


def _build_in_maps(inputs):
    return _prep(**inputs)[0]


def _gather(results, bo_p):
    out = np.empty((B, S, N_OUT), np.float32)
    for b in range(B):
        # outT [128, m2, iq, j, n]: out dim = (2*m2+j)*128 + part,
        # seq = iq*NMM + n
        oT = (results[2 * b]["outT"].astype(np.float32)
              + results[2 * b + 1]["outT"].astype(np.float32))
        oT = oT.transpose(1, 3, 0, 2, 4).reshape(N_OUT, S)
        out[b] = oT.T + bo_p
    return out


def kernel(queries, keys, values, Wq, bq, Wk, bk, Wv, bv, Wo, bo):
    from concourse.bass_utils import run_bass_kernel_spmd

    in_maps, bo_p = _prep(queries, keys, values, Wq, bq, Wk, bk, Wv, bv, Wo, bo)
    if "nc" not in _CACHE:
        _CACHE["nc"] = _build_nc()
    res = run_bass_kernel_spmd(_CACHE["nc"], in_maps, core_ids=list(range(8)))
    return _gather(res.results, bo_p)


# revision 25
# speedup vs baseline: 1.0143x; 1.0143x over previous
"""InterpretableMultiHeadAttention on 8 Trainium2 NeuronCores (Bass/Tile).

Sharding: core c -> batch b = c//2, head-group hg = c%2 (8 of 16 heads).
Math folding (exact up to fp rounding):
  v' = v @ Wv.T + bv, x = sum_h attn_h @ v'_h, out = x @ Wo.T + bo
  Since softmax rows sum to 1:  attn @ (1 bv^T) = 1 bv^T, so
  out = (sum_h attn_h @ v_h) @ (Wo @ Wv).T + (H * Wo @ bv + bo)
The 1/sqrt(d) score scale folds into Wq/bq.

Schedule per core: fp8 DoubleRow projections (contraction 256/step);
per query-block of 512: scores^T with 2-head row-packing (fp16, d=64
contraction via PE quadrants), exp split between ScalarE (table exp ->
fp8) and the DVE (one-op Schraudolph bit-trick exp -> fp8), PV matmuls
in fp8 DoubleRow over 256-key windows with a per-head ones-column at
64+h so softmax denominators accumulate on PSUM rows 64..71. Block
epilogue: 3-level den tree-sum, fast-approx reciprocal, PE-matmul
broadcast of 1/den to 64 partitions, divide + head tree-sum on DVE,
Wov projection, fp16 output DMA. Host sums the two half-head partial
projections per batch and adds the bias.
"""
import numpy as np
import ml_dtypes

N_OUT = 1024
N_HEADS = 16
D_K = 64
B = 4
S = 2048
FC8 = 4         # 1024 contraction as 4 DoubleRow chunks of 256
PAIRS = 4       # 8 local heads as 4 row-packed pairs
NMM = 512       # matmul moving free dim
JC = S // 128   # 16 key chunks of 128
JCP = JC // 2   # 8 key chunk-pairs of 256 (DoubleRow PV)
IQ = S // NMM   # 4 query blocks of 512
MV = 80         # PV lhsT cols: 64 v dims + ones at 64+h + pad to 16B step

# Schraudolph exp: fp8e4m3 bits = round(A8*s + B8); minimax C over [-3,3]
A8 = 8.0 * 1.4426950408889634
B8 = 56.0 - 0.370

# chunk (p, jc) exp routed to DVE iff (jc + p) % 3 == 0  (22 of 64)
def _dve_chunk(p, jc):
    return (jc + p) % 3 == 0

_CACHE = {}


def _build_nc(debug=False):
    from contextlib import ExitStack
    import concourse.bass as bass
    import concourse.bacc as bacc
    import concourse.tile as tile
    import concourse.mybir as mybir

    f16 = mybir.dt.float16
    f32 = mybir.dt.float32
    f32r = mybir.dt.float32r
    f8 = mybir.dt.float8e4
    i8 = mybir.dt.int8
    DR = mybir.MatmulPerfMode.DoubleRow

    nc = bacc.Bacc("TRN2", target_bir_lowering=False, debug=False, num_devices=8)

    xq_d = nc.dram_tensor("xq", [128, FC8, 2, S], f8, kind="ExternalInput")
    xk_d = nc.dram_tensor("xk", [128, FC8, 2, S], f8, kind="ExternalInput")
    wq_d = nc.dram_tensor("wq", [128, FC8, 2, 512], f8, kind="ExternalInput")
    wk_d = nc.dram_tensor("wk", [128, FC8, 2, 512], f8, kind="ExternalInput")
    bq_d = nc.dram_tensor("bq", [128, PAIRS], f32, kind="ExternalInput")
    bk_d = nc.dram_tensor("bk", [128, PAIRS], f32, kind="ExternalInput")
    vv_d = nc.dram_tensor("vv", [PAIRS, 128, JCP, 2, 2, MV], f8, kind="ExternalInput")
    wov_d = nc.dram_tensor("wov", [64, N_OUT], f16, kind="ExternalInput")
    sel_d = nc.dram_tensor("sel", [128, 8, 64], f16, kind="ExternalInput")
    out_d = nc.dram_tensor("outT", [128, 4, IQ, 2, NMM], f16,
                           kind="ExternalOutput")
    if debug:
        dbg_qT = nc.dram_tensor("dbg_qT", [128, 4, 512], f16, kind="ExternalOutput")
        dbg_kT = nc.dram_tensor("dbg_kT", [128, 4, 512], f16, kind="ExternalOutput")
        dbg_ep = nc.dram_tensor("dbg_ep", [128, 2, 2, NMM], f16, kind="ExternalOutput")
        dbg_yb = nc.dram_tensor("dbg_yb", [72, 8, NMM], f16, kind="ExternalOutput")
        dbg_dn = nc.dram_tensor("dbg_dn", [128, NMM], f32, kind="ExternalOutput")
        dbg_y16 = nc.dram_tensor("dbg_y16", [64, 512], f16, kind="ExternalOutput")

    with tile.TileContext(nc) as tc, ExitStack() as ctx:
        const = ctx.enter_context(tc.tile_pool(name="const", bufs=1))
        qkall = ctx.enter_context(tc.tile_pool(name="qkall", bufs=1))
        epool = ctx.enter_context(tc.tile_pool(name="epool", bufs=6))
        blk = ctx.enter_context(tc.tile_pool(name="blk", bufs=2))
        fin = ctx.enter_context(tc.tile_pool(name="fin", bufs=2))
        ost_p = ctx.enter_context(tc.tile_pool(name="ost_p", bufs=2))
        ps_mm = ctx.enter_context(tc.tile_pool(name="ps_mm", bufs=3, space="PSUM"))
        ps_y = ctx.enter_context(tc.tile_pool(name="ps_y", bufs=2, space="PSUM"))
        xctx = ExitStack()
        xstage = xctx.enter_context(tc.tile_pool(name="xstage", bufs=1))

        # ---- input loads (proj-critical first) ----
        wq_sb = const.tile([128, FC8, 2, 512], f8, tag="wq")
        nc.sync.dma_start(out=wq_sb[:], in_=wq_d[:])
        bq_sb = const.tile([128, PAIRS], f32, tag="bq")
        nc.sync.dma_start(out=bq_sb[:], in_=bq_d[:])
        bk_sb = const.tile([128, PAIRS], f32, tag="bk")
        nc.gpsimd.dma_start(out=bk_sb[:], in_=bk_d[:])
        wk_sb = const.tile([128, FC8, 2, 512], f8, tag="wk")
        nc.scalar.dma_start(out=wk_sb[:], in_=wk_d[:])
        engs = [nc.sync, nc.scalar, nc.gpsimd]
        xq_sb = xstage.tile([128, FC8, 2, S], f8, tag="xq")
        xk_sb = xstage.tile([128, FC8, 2, S], f8, tag="xk")
        ei = 0
        for fc in range(FC8):
            engs[ei % 3].dma_start(out=xq_sb[:, fc], in_=xq_d[:, fc])
            ei += 1
            engs[ei % 3].dma_start(out=xk_sb[:, fc], in_=xk_d[:, fc])
            ei += 1
        vv_sb = []
        for p in range(PAIRS):
            t = qkall.tile([128, JCP, 2, 2, MV], f8, tag=f"vv{p}")
            nc.sync.dma_start(out=t[:], in_=vv_d[p])
            vv_sb.append(t)
        wov_sb = const.tile([64, N_OUT], f16, tag="wov")
        nc.sync.dma_start(out=wov_sb[:], in_=wov_d[:])
        sel_sb = const.tile([128, 8, 64], f16, tag="sel")
        nc.gpsimd.dma_start(out=sel_sb[:], in_=sel_d[:])
        den32 = const.tile([128, NMM], f32, tag="den32")
        nc.vector.memset(den32[:], 1.0)
        rec32 = const.tile([128, NMM], f32, tag="rec32")

        qT, kT = {}, {}

        def _proj_quarter(p, qk, sc2):
            dst = (qT if qk == 0 else kT)[p]
            w_sb = wq_sb if qk == 0 else wk_sb
            b_sb = bq_sb if qk == 0 else bk_sb
            x_sb = xq_sb if qk == 0 else xk_sb
            ps = ps_mm.tile([128, 2, 512], f32, tag="mm", name="projps")
            for half in range(2):
                c0 = (sc2 * 2 + half) * 512
                for fc in range(FC8):
                    nc.tensor.matmul(
                        out=ps[:, half, :],
                        lhsT=w_sb[:, fc, :, p * 128:(p + 1) * 128],
                        rhs=x_sb[:, fc, :, c0:c0 + 512],
                        start=(fc == 0),
                        stop=(fc == FC8 - 1),
                        perf_mode=DR,
                    )
            if qk == 0:
                nc.scalar.add(
                    out=dst[:, sc2 * 2:sc2 * 2 + 2, :],
                    in_=ps[:],
                    add=b_sb[:, p:p + 1],
                )
            else:
                nc.vector.tensor_scalar_add(
                    out=dst[:, sc2 * 2:sc2 * 2 + 2, :],
                    in0=ps[:],
                    scalar1=b_sb[:, p:p + 1],
                )

        def proj_closures(p):
            qT[p] = qkall.tile([128, 4, 512], f16, tag=f"qT{p}", name=f"qT{p}")
            kT[p] = qkall.tile([128, 4, 512], f16, tag=f"kT{p}", name=f"kT{p}")
            return [lambda qk=qk, sc2=sc2: _proj_quarter(p, qk, sc2)
                    for sc2 in range(2) for qk in range(2)]

        def emit_fin_rb(st, p):
            """rb2 = PE broadcast of 1/den for heads 2p, 2p+1."""
            rb2 = ps_mm.tile([128, 2, 512], f32, tag="mm", name=f"rb{p}")
            for j in range(2):
                nc.tensor.matmul(
                    out=rb2[0:64, j, :],
                    lhsT=sel_sb[64:72, 2 * p + j, :],
                    rhs=st["den16"][64:72, :],
                    start=True, stop=True,
                    tile_position=(64, 0),
                )
            st[f"rb{p}"] = rb2

        def emit_fin_mul(st, p):
            if f"rb{p}" not in st:
                emit_fin_rb(st, p)
            rb2 = st[f"rb{p}"]
            m = fin.tile([64, 2, 512], f16, tag=f"m{p}", name=f"m{p}")
            if st["last"]:
                # tail path: stage 1/den via the otherwise-idle ScalarE so
                # the DVE multiply runs at 2x (both operands fp16 SBUF)
                rb16 = fin.tile([64, 2, 512], f16, tag=f"rb16_{p}",
                                name=f"rb16_{p}")
                nc.scalar.copy(out=rb16[:], in_=rb2[0:64, :, :])
                nc.vector.tensor_mul(
                    out=m[:], in0=st["y_blk"][0:64, 2 * p:2 * p + 2, :],
                    in1=rb16[:])
            else:
                nc.vector.tensor_mul(
                    out=m[:], in0=st["y_blk"][0:64, 2 * p:2 * p + 2, :],
                    in1=rb2[0:64, :, :])
            st[f"m{p}"] = m

        def emit_fin_add(st, k):
            t = fin.tile([64, 2, 512], f16, tag=f"t{k}", name=f"t{k}")
            nc.vector.tensor_add(
                out=t[:], in0=st[f"m{2 * k}"][:], in1=st[f"m{2 * k + 1}"][:])
            st[f"t{k}"] = t

        def emit_fin_y16(st):
            tt = fin.tile([64, 2, 512], f16, tag="tt")
            nc.vector.tensor_add(out=tt[:], in0=st["t0"][:], in1=st["t1"][:])
            y16 = blk.tile([64, 512], f16, tag="y16")
            nc.vector.tensor_add(out=y16[:], in0=tt[:, 0, :], in1=tt[:, 1, :])
            st["y16"] = y16

        def emit_out(st, m2):
            po = ps_mm.tile([128, 2, 512], f32, tag="mm", name=f"po{m2}")
            for j in range(2):
                nc.tensor.matmul(
                    out=po[:, j, :],
                    lhsT=wov_sb[:, (2 * m2 + j) * 128:(2 * m2 + j + 1) * 128],
                    rhs=st["y16"][:],
                    start=True, stop=True,
                )
            ost = ost_p.tile([128, 2, NMM], f16, tag="ost")
            if st["last"] and m2 % 2 == 0:
                nc.scalar.copy(out=ost[:], in_=po[:])
            else:
                nc.vector.tensor_copy(out=ost[:], in_=po[:])
            eng = nc.sync if m2 % 2 == 0 else nc.gpsimd
            eng.dma_start(out=out_d[:, m2, st["iq"], :, :], in_=ost[:])

        def fin_closures(st, drain=False):
            ops = []
            if drain:
                # front-load the PE broadcasts so the DVE divide chain
                # never waits on a matmul mid-stream
                ops += [lambda p=p: emit_fin_rb(st, p) for p in range(PAIRS)]
            ops += [lambda p=p: emit_fin_mul(st, p) for p in range(PAIRS)]
            ops += [lambda k=k: emit_fin_add(st, k) for k in range(2)]
            ops.append(lambda: emit_fin_y16(st))
            ops += [lambda m2=m2: emit_out(st, m2) for m2 in range(4)]
            return ops

        prev = None         # epilogue state dict of the previous block
        bops = []           # spread-out boundary ops, one per chunk-pair
        pend_pv = None      # deferred PV emission (one chunk-pair behind)
        for iq in range(IQ):
            y_blk = blk.tile([72, 8, NMM], f16, tag="yblk")
            den_acc = blk.tile([128, 2, NMM], f16, tag="dacc")
            for p in range(PAIRS):
                if iq == 0:
                    if p == 0:
                        p0ops = proj_closures(0)
                        for op in p0ops[:2]:
                            op()
                        bops = p0ops[2:] + proj_closures(1)
                    elif p < PAIRS - 1:
                        bops = bops + proj_closures(p + 1)
                if p == 1 and prev is not None:
                    bops = bops + fin_closures(prev)
                    prev = None
                yP = [ps_y.tile([MV, NMM], f32, tag="yab", name=f"yP{a}")
                      for a in range(2)]

                def emit_pv(p, jcp, ep, yP):
                    for a in range(2):
                        nc.tensor.matmul(
                            out=yP[a][:],
                            lhsT=vv_sb[p][:, jcp, a, :, :],
                            rhs=ep[:, a, :, :],
                            start=(jcp == 0), stop=(jcp == JCP - 1),
                            perf_mode=DR,
                            skip_group_check=True,
                        )
                    if jcp == JCP - 1:
                        # drain this pair: numerators + den rows to SBUF
                        for a in range(2):
                            nc.vector.tensor_copy(
                                out=y_blk[:, 2 * p + a, :], in_=yP[a][0:72, :])
                        if p == 0:
                            nc.vector.tensor_copy(
                                out=den_acc[64:72], in_=y_blk[64:72, 0:2, :])
                        else:
                            nc.vector.tensor_add(
                                out=den_acc[64:72], in0=den_acc[64:72],
                                in1=y_blk[64:72, 2 * p:2 * p + 2, :])

                for jcp in range(JCP):
                    ep = epool.tile([128, 2, 2, NMM], f8, tag="e")
                    for ko in range(2):
                        jc = 2 * jcp + ko
                        sAB = ps_mm.tile([128, 2, 512], f32, tag="mm")
                        nc.tensor.matmul(
                            out=sAB[:, 0, :],
                            lhsT=kT[p][0:64, jc // 4, (jc % 4) * 128:(jc % 4) * 128 + 128],
                            rhs=qT[p][0:64, iq, :],
                            start=True, stop=True,
                            tile_position=(0, 0),
                        )
                        nc.tensor.matmul(
                            out=sAB[:, 1, :],
                            lhsT=kT[p][64:128, jc // 4, (jc % 4) * 128:(jc % 4) * 128 + 128],
                            rhs=qT[p][64:128, iq, :],
                            start=True, stop=True,
                            tile_position=(64, 0),
                        )
                        if _dve_chunk(p, jc):
                            nc.vector.tensor_scalar(
                                out=ep.bitcast(i8)[:, :, ko, :],
                                in0=sAB[:],
                                scalar1=A8, scalar2=B8,
                                op0=mybir.AluOpType.mult,
                                op1=mybir.AluOpType.add,
                            )
                        else:
                            nc.scalar.activation(
                                out=ep[:, :, ko, :], in_=sAB[:],
                                func=mybir.ActivationFunctionType.Exp,
                            )
                    if pend_pv is not None:
                        pend_pv()
                    pend_pv = (lambda p=p, jcp=jcp, ep=ep, yP=yP:
                               emit_pv(p, jcp, ep, yP))
                    if bops:
                        bops.pop(0)()
            if pend_pv is not None:
                pend_pv()
                pend_pv = None
            if iq == 0:
                xctx.close()  # release x staging after last projection

            # block-end denominator fold + fast reciprocal (partitions 64..71;
            # reciprocal_approx_fast mishandles base_partition != 0, so it
            # runs full-tile over the once-memset background)
            nc.vector.tensor_add(
                out=den32[64:72], in0=den_acc[64:72, 0, :],
                in1=den_acc[64:72, 1, :])
            nc.vector.reciprocal_approx_fast(out=rec32[:, :], in_=den32[:, :])
            den16 = blk.tile([128, NMM], f16, tag="den16")
            nc.vector.tensor_copy(out=den16[64:72, :], in_=rec32[64:72, :])
            prev = {"iq": iq, "y_blk": y_blk, "den16": den16,
                    "last": iq == IQ - 1}
        # drain the last block
        for op in fin_closures(prev, drain=True):
            op()

    nc.compile()
    return nc


def _prep(queries, keys, values, Wq, bq, Wk, bk, Wv, bv, Wo, bo):
    """Host-side sharding/layout prep. Returns (in_maps, bo_p)."""
    queries = np.asarray(queries, np.float32)
    keys = np.asarray(keys, np.float32)
    values = np.asarray(values, np.float32)
    Wq = np.asarray(Wq, np.float32)
    bq = np.asarray(bq, np.float32)
    Wk = np.asarray(Wk, np.float32)
    bk = np.asarray(bk, np.float32)
    Wv = np.asarray(Wv, np.float32)
    bv = np.asarray(bv, np.float32)
    Wo = np.asarray(Wo, np.float32)
    bo = np.asarray(bo, np.float32)
    f8 = ml_dtypes.float8_e4m3

    scale = 1.0 / np.sqrt(np.float32(D_K))
    Wq_s = Wq * scale
    bq_s = bq * scale
    Wov = Wo @ Wv                       # [1024, 64]
    bo_p = bo + N_HEADS * (Wo @ bv)     # [1024]
    wov_h = np.ascontiguousarray(Wov.T.astype(np.float16))  # [64, 1024]

    # sel[64+r, h, :]: ones row used to broadcast recip row r=h to 64 parts
    sel = np.zeros((128, 8, 64), np.float16)
    for h in range(8):
        sel[64 + h, h, :] = 1.0

    def contr_fold(a):
        # [1024, cols] -> [128, 4, 2, cols]: contraction g = fc*256+j*128+k
        cols = a.shape[1]
        return np.ascontiguousarray(
            a.reshape(FC8, 2, 128, cols).transpose(2, 0, 1, 3).astype(f8))

    in_maps = []
    for c in range(8):
        b = c // 2
        hg = c % 2
        hsl = slice(hg * 512, (hg + 1) * 512)
        xq = contr_fold(queries[b].T)
        xk = contr_fold(keys[b].T)
        wq = contr_fold(Wq_s[hsl].T)
        wk = contr_fold(Wk[hsl].T)
        bq_c = np.ascontiguousarray(bq_s[hsl].reshape(PAIRS, 128).T)
        bk_c = np.ascontiguousarray(bk[hsl].reshape(PAIRS, 128).T)
        # vv [PAIRS, 128, JCP, 2(a), 2(ko), MV]; key = jcp*256 + ko*128 + k
        vb = values[b][:, hsl].reshape(JCP, 2, 128, 8, 64)  # [jcp, ko, k, h, d]
        vv = np.zeros((128, JCP, 8, 2, MV), f8)
        vv[:, :, :, :, :64] = vb.transpose(2, 0, 3, 1, 4).astype(f8)
        for h in range(8):
            vv[:, :, h, :, 64 + h] = 1.0
        vv = vv.reshape(128, JCP, PAIRS, 2, 2, MV).transpose(2, 0, 1, 3, 4, 5)
        in_maps.append({
            "xq": xq, "xk": xk, "wq": wq, "wk": wk,
            "bq": bq_c, "bk": bk_c, "vv": np.ascontiguousarray(vv),
            "wov": wov_h, "sel": sel,
        })
    return in_maps, bo_p


def _build_in_maps(inputs):
    return _prep(**inputs)[0]


def _gather(results, bo_p):
    out = np.empty((B, S, N_OUT), np.float32)
    for b in range(B):
        # outT [128, m2, iq, j, n]: out dim = (2*m2+j)*128 + part,
        # seq = iq*NMM + n
        oT = (results[2 * b]["outT"].astype(np.float32)
              + results[2 * b + 1]["outT"].astype(np.float32))
        oT = oT.transpose(1, 3, 0, 2, 4).reshape(N_OUT, S)
        out[b] = oT.T + bo_p
    return out


def kernel(queries, keys, values, Wq, bq, Wk, bk, Wv, bv, Wo, bo):
    from concourse.bass_utils import run_bass_kernel_spmd

    in_maps, bo_p = _prep(queries, keys, values, Wq, bq, Wk, bk, Wv, bv, Wo, bo)
    if "nc" not in _CACHE:
        _CACHE["nc"] = _build_nc()
    res = run_bass_kernel_spmd(_CACHE["nc"], in_maps, core_ids=list(range(8)))
    return _gather(res.results, bo_p)
